# revision 27
# baseline (speedup 1.0000x reference)
"""Gaussian-kernel attention for Trainium2 (Bass/Tile), 8-core data-parallel.

Computes out = x + K @ x with K = exp(-r * d2), d2[t,s] = ||x_t - x_s||^2,
per batch.  Decomposition used on-chip:

    d2 = sq_t + sq_s - 2*G          (G = X X^T, sq = rowwise |x|^2)
    K  = e_t * exp(2r*G) * e_s      (e_i = exp(-r*sq_i))
    out[t] = x[t] + e_t * sum_s [exp(2r*G - r*sq_s)][s,t] * x[s]

The e_s factor folds into the ACT exp as a per-partition bias, so mm2's
stationary operand is plain bf16 x.  Structure per core (4 batches):

  phase 1: all 4 batch prologues up front (x load, row stats, X^T via a
     DRAM round-trip DMA-xbar transpose) so the main loops never wait on
     the sync queue at batch boundaries.
  phase 2: one flat pipeline over (batch, t-block) pairs; per 1024-col
     t-block an s-loop of
       mm1 (G = X X^T, two row-group-packed N=512 matmuls, one s ahead)
       -> ACT exp (the only T^2 elementwise op, ~1.1us per tile; the
          critical-path engine, ~saturated)
       -> mm2 (outT += x^T A, two column-group-packed N=512 matmuls;
          chunk n lands on PSUM partitions n*64..n*64+63).
     The epilogue (outT -> out) PE-transposes 128-col chunks; those 8
     transposes + rescales drip through the NEXT block's s-loop so they
     ride the PE/DVE slack instead of stalling the ACT stream.

Sharding: pure data-parallel over batch B=32 -> 4 batches per core x 8 cores.
"""

import os
import sys

import numpy as np

sys.path.insert(0, "/opt/trn_rl_repo")

import concourse.bass as bass
import concourse.tile as tile
from concourse import bacc, masks, mybir
from concourse.bass_utils import run_bass_kernel_spmd

FP32 = mybir.dt.float32
BF16 = mybir.dt.bfloat16

B, T, C = 32, 2048, 64
N_CORES = 8
BPC = B // N_CORES  # batches per core

# Stashed by kernel() for the test harness (exec time etc.)
LAST_RESULTS = None


def _body(ctx, tc, out_ap, x_ap, r, bpc, t, dbg=False):
    """Emit the per-core kernel IR.

    out_ap/x_ap: DRAM APs of shape [bpc, t, C].
    r: python float (r_sigma value, baked as immediates).
    """
    nc = tc.nc

    def dump(name, sb_ap, dt=None):
        if not dbg:
            return
        d = nc.dram_tensor(
            name, list(sb_ap.shape), dt or sb_ap.dtype, kind="ExternalOutput"
        ).ap()
        nc.sync.dma_start(out=d, in_=sb_ap)

    nt = t // 128          # 128-row s/t blocks
    tblk = min(1024, t)    # t-block width processed per G tile (<= 2 psum banks)
    ntb = t // tblk        # outer t-block count
    kpb = tblk // 128      # 128-col sub-blocks per t-block

    exp2r = 2.0 * r

    # SBUF pools; per-batch tiles (x32/xt/stats) stay live all kernel
    xpool = ctx.enter_context(tc.tile_pool(name="x32", bufs=1))
    xxpool = ctx.enter_context(tc.tile_pool(name="xx", bufs=2))
    sqpool = ctx.enter_context(tc.tile_pool(name="sq", bufs=1))
    xbpool = ctx.enter_context(tc.tile_pool(name="xbp", bufs=2))
    xtpool = ctx.enter_context(tc.tile_pool(name="xt", bufs=1))
    idpool = ctx.enter_context(tc.tile_pool(name="idp", bufs=1))
    apool = ctx.enter_context(tc.tile_pool(name="a0", bufs=3))
    opool = ctx.enter_context(tc.tile_pool(name="osb", bufs=2))
    # PSUM (8 banks): G = [128, tblk] fp32 x2 bufs (4), P = [128, tblk] x1
    # (2), trp = [128, kpb, 128] bf16 x2 (2)
    gpool = ctx.enter_context(tc.tile_pool(name="gps", bufs=2, space="PSUM"))
    ppool = ctx.enter_context(tc.tile_pool(name="pps", bufs=1, space="PSUM"))
    tpool = ctx.enter_context(tc.tile_pool(name="tps", bufs=2, space="PSUM"))
    # DRAM scratch for the bf16 transpose round-trip
    dpool = ctx.enter_context(tc.tile_pool(name="dsc", bufs=2, space="DRAM"))

    ident = idpool.tile([128, 128], BF16)
    masks.make_identity(nc, ident[:])

    # Dependency-free dummy exp: pulls the ~1.3us ACT_TABLE_LOAD (lazily
    # inserted before the first ACTIVATE) into the startup dead time instead
    # of the head of the real exp stream.
    warm = idpool.tile([128, 32], FP32, name="warm")
    nc.gpsimd.memset(warm[:], 0.0)
    nc.scalar.activation(
        warm[:, 16:32], warm[:, 0:16], mybir.ActivationFunctionType.Exp
    )

    # Degree-6 polynomial fit of ev(s) = exp(-r*s) over the attainable range
    # of s = |x|^2 (chi^2_64; [0, 280] covers it with huge margin), weighted
    # for relative error.  Evaluated on the DVE so no ACT instruction ever
    # precedes the big exp stream (the scheduler hoists early ACT ops ahead
    # of it and stalls the whole pipeline on their inputs).
    s_grid = np.linspace(0.0, 280.0, 2048)
    vals = np.exp(-r * s_grid)
    cf = np.polyfit(s_grid, vals, 6, w=1.0 / vals)  # highest power first

    # ---- prologues (emitted just-in-time so their DMAs and ev-exps don't
    # head-of-line-block the sync/ACT queues at startup) ----
    x32s, sqs, evs, msqs, xts = [], [], [], [], []

    def prologue(b):
        xb_dram = x_ap[b].rearrange("(k p) c -> p k c", p=128)   # [128, nt, C]

        x32 = xpool.tile([128, nt, C], FP32, tag=f"x32_{b}")
        # batch 0's load must reach SBUF ASAP; later batches' loads go on
        # the idle gpsimd (SWDGE) queue so the scheduler can't hoist them
        # ahead of it on the sync queue.
        if b == 0:
            nc.sync.dma_start(out=x32[:], in_=xb_dram)
        else:
            nc.gpsimd.dma_start(out=x32[:], in_=xb_dram)
        xt = xtpool.tile([128, t + nt * C], BF16, tag=f"xt{b}")
        # xt[c, tt] = xt[64+c, tt] = x[tt, c] for c < 64; cols [t, t+nt*C)
        # hold bf16 x with s on partitions (mm2's stationary operand).

        def stats():
            xx = xxpool.tile([128, nt, C], FP32, name="xx")
            nc.vector.tensor_mul(xx[:], x32[:], x32[:])
            sq = sqpool.tile([128, nt], FP32, name="sq", tag=f"sq{b}")
            nc.vector.tensor_reduce(
                sq[:], xx[:], axis=mybir.AxisListType.X, op=mybir.AluOpType.add
            )
            # per-partition exp bias: -r*|x_s|^2 (folds e_s into the exp)
            msq = sqpool.tile([128, nt], FP32, name="msq", tag=f"msq{b}")
            nc.vector.tensor_scalar_mul(msq[:], sq[:], -r)
            # ev = exp(-r*sq) via DVE Horner (8 tiny ops, no ACT involved)
            t0 = sqpool.tile([128, nt], FP32, name="t0", tag=f"pa{b % 2}")
            t1 = sqpool.tile([128, nt], FP32, name="t1", tag=f"pb{b % 2}")
            nc.vector.tensor_scalar(
                t0[:], sq[:], 0.0, float(cf[0]),
                op0=mybir.AluOpType.mult, op1=mybir.AluOpType.add,
            )
            cur, nxt = t0, t1
            for a in (0.0, cf[1], cf[2], cf[3], cf[4], cf[5]):
                nc.vector.scalar_tensor_tensor(
                    nxt[:], in0=cur[:], scalar=float(a), in1=sq[:],
                    op0=mybir.AluOpType.add, op1=mybir.AluOpType.mult,
                )
                cur, nxt = nxt, cur
            ev = sqpool.tile([128, nt], FP32, name="ev", tag=f"ev{b}")
            nc.vector.tensor_scalar_add(ev[:], cur[:], float(cf[6]))
            return sq, msq, ev

        if b == 0:
            # Batch 0 gates the whole exp stream, and its DMA-xbar round
            # trip sits ~10us behind DVE stats + queue hoisting.  Hybrid:
            # build X^T for k-blocks 0..7 (needed by the first t-window and
            # the early s-weights) with PE transposes + DVE copies, and let
            # blocks 8..15 arrive via the DMA round trip (pure DMA work,
            # reading the bf16 staging copy; not needed for ~15us).
            xb16 = xbpool.tile([128, nt, C], BF16, tag="xb16")
            nc.vector.tensor_copy(xb16[:], x32[:])
            xbd0 = dpool.tile([t // 2, 2 * C], BF16, name="xbd0")
            xbdv = xbd0.rearrange("(k p) c -> p k c", p=128)
            nc.sync.dma_start(out=xbdv[:, :, 0:C], in_=xb16[:, nt // 2 :])
            nc.sync.dma_start(out=xbdv[:, :, C : 2 * C], in_=xb16[:, nt // 2 :])
            nc.sync.dma_start_transpose(out=xt[:, t // 2 : t], in_=xbd0[:])
            for k in range(nt // 2):
                trq = tpool.tile([128, 128], BF16, name="trpj")
                nc.tensor.transpose(trq[0:64, :], xb16[:, k], ident[:])
                nc.vector.tensor_copy(
                    xt[0:64, k * 128 : (k + 1) * 128], trq[0:64, :]
                )
                nc.vector.tensor_copy(
                    xt[64:128, k * 128 : (k + 1) * 128], trq[0:64, :]
                )
            sq, msq, ev = stats()
            nc.vector.tensor_copy(
                xt[:, t : t + nt * C].rearrange("p (k c) -> p k c", c=C),
                xb16[:],
            )
        else:
            sq, msq, ev = stats()
            # bf16 x written TWICE side by side so one full-width DMA-xbar
            # transpose yields X^T duplicated on both partition halves (row
            # groups for packed mm1); hidden under earlier batches' compute.
            xbp = xbpool.tile([128, nt, 2 * C], BF16, name="xbp")
            nc.vector.tensor_copy(xbp[:, :, 0:C], x32[:])
            nc.vector.tensor_copy(xbp[:, :, C : 2 * C], x32[:])
            xbd = dpool.tile([t, 2 * C], BF16, name="xbd")
            nc.sync.dma_start(
                out=xbd.rearrange("(k p) c -> p k c", p=128), in_=xbp[:]
            )
            nc.sync.dma_start_transpose(out=xt[:, 0:t], in_=xbd[:])
            nc.vector.tensor_copy(
                xt[:, t : t + nt * C].rearrange("p (k c) -> p k c", c=C),
                xbp[:, :, 0:C],
            )
        x32s.append(x32)
        sqs.append(sq)
        evs.append(ev)
        msqs.append(msq)
        xts.append(xt)
        if dbg and b == 0:
            dump("dbg_sq", sq[:])

    # ---- main: flat pipeline over (batch, t-block) ----
    blocks = [(b, ti) for b in range(bpc) for ti in range(ntb)]

    def mm1(b, ti, s):
        xt = xts[b]
        g = gpool.tile([128, tblk], FP32, name="g_ps", tag="g")
        for n in range(2):
            base = 64 * n
            nc.tensor.matmul(
                g[:, n * 512 : (n + 1) * 512],
                lhsT=xt[base : base + 64, s * 128 : (s + 1) * 128],
                rhs=xt[
                    base : base + 64,
                    ti * tblk + n * 512 : ti * tblk + (n + 1) * 512,
                ],
                start=True,
                stop=True,
                tile_position=(base, 0),
            )
        return g

    pend = None  # previous block's epilogue: dict(ot, osb, b, ti)

    def drip(j):
        ot = pend["ot"]
        pb, pti = pend["b"], pend["ti"]
        # per-j trp tiles from a 2-buffer pool: adjacent transposes land in
        # different PSUM banks, so transpose j+1 (start=True clears its
        # bank) doesn't serialize against j's reader.
        trp = tpool.tile([128, 128], BF16, name="trpj")
        nc.tensor.transpose(trp[:], ot[:, j * 128 : (j + 1) * 128], ident[:])
        k = pti * kpb + j
        half = (j // (kpb // 2)) * 64
        nc.vector.scalar_tensor_tensor(
            pend["osb"][:, j],
            in0=trp[:, half : half + 64],
            scalar=evs[pb][:, k : k + 1],
            in1=x32s[pb][:, k],
            op0=mybir.AluOpType.mult,
            op1=mybir.AluOpType.add,
        )

    def flush():
        pb, pti = pend["b"], pend["ti"]
        ob_dram = out_ap[pb].rearrange("(k p) c -> p k c", p=128)
        nc.sync.dma_start(
            out=ob_dram[:, pti * kpb : (pti + 1) * kpb], in_=pend["osb"][:]
        )

    def tail_stt(osb, trp, tb, tti, j):
        k = tti * kpb + j
        half = (j // (kpb // 2)) * 64
        nc.vector.scalar_tensor_tensor(
            osb[:, j],
            in0=trp[:, half : half + 64],
            scalar=evs[tb][:, k : k + 1],
            in1=x32s[tb][:, k],
            op0=mybir.AluOpType.mult,
            op1=mybir.AluOpType.add,
        )

    prologue(0)
    prologue(1)
    g_cur = mm1(*blocks[0], 0)
    for bi, (b, ti) in enumerate(blocks):
        if bi in (1, 2) and bi + 1 < bpc:
            prologue(bi + 1)
        msq, xt = msqs[b], xts[b]
        yw = xt[:, t : t + nt * C].rearrange("p (k c) -> p k c", c=C)
        p_ps = ppool.tile([128, tblk], FP32)

        for s in range(nt):
            if s + 1 < nt:
                g_next = mm1(b, ti, s + 1)
            elif bi + 1 < len(blocks):
                g_next = mm1(*blocks[bi + 1], 0)
            else:
                g_next = None
            a0 = apool.tile([128, tblk], BF16)
            nc.scalar.activation(
                a0[:],
                g_cur[:],
                mybir.ActivationFunctionType.Exp,
                scale=exp2r,
                bias=msq[:, s : s + 1],
            )
            if dbg and b == 0 and ti == 0 and s == 0:
                gsb = xxpool.tile([128, tblk], FP32, tag="gdump")
                nc.vector.tensor_copy(gsb[:], g_cur[:])
                dump("dbg_g00", gsb[:])
                dump("dbg_a00", a0[:])
            # mm2: two concurrent column-group matmuls; chunk n -> PSUM
            # partitions [n*64, n*64+64), cols [n*512, n*512+512).
            for n in range(2):
                nc.tensor.matmul(
                    p_ps[n * 64 : (n + 1) * 64, n * 512 : (n + 1) * 512],
                    lhsT=yw[:, s],
                    rhs=a0[:, n * 512 : (n + 1) * 512],
                    start=(s == 0),
                    stop=(s == nt - 1),
                    skip_group_check=True,
                    tile_position=(0, n * 64),
                )
            # drip the previous block's epilogue through this block's slack
            if pend is not None:
                if s % 2 == 0 and s // 2 < kpb:
                    drip(s // 2)
                if s == 2 * kpb - 1:
                    flush()
                    pend = None
            g_cur = g_next

        if bi == len(blocks) - 1:
            # final block: its epilogue is fully exposed — pipeline it at
            # 128-col granularity (chunk CAST -> PE transpose -> rescale)
            # instead of a serial bulk CAST followed by the drip chain.
            otl = opool.tile([128, tblk], BF16, tag="ot")
            osb = opool.tile([128, kpb, C], FP32, name="osb", tag="osb")
            ob_dram = out_ap[b].rearrange("(k p) c -> p k c", p=128)
            trps = []
            for j in range(kpb):
                nc.vector.tensor_copy(
                    otl[:, j * 128 : (j + 1) * 128],
                    p_ps[:, j * 128 : (j + 1) * 128],
                )
                trp = tpool.tile([128, 128], BF16, name="trpj")
                nc.tensor.transpose(
                    trp[:], otl[:, j * 128 : (j + 1) * 128], ident[:]
                )
                trps.append(trp)
                if j >= 1:
                    tail_stt(osb, trps[j - 1], b, ti, j - 1)
            tail_stt(osb, trps[kpb - 1], b, ti, kpb - 1)
            nc.sync.dma_start(
                out=ob_dram[:, ti * kpb : (ti + 1) * kpb], in_=osb[:]
            )
        else:
            # stage this block's epilogue
            ot = opool.tile([128, tblk], BF16, tag="ot")
            nc.vector.tensor_copy(ot[:], p_ps[:])
            osb = opool.tile([128, kpb, C], FP32, name="osb", tag="osb")
            pend = {"ot": ot, "osb": osb, "b": b, "ti": ti}


def build(r, bpc=BPC, t=T, dbg=False):
    """Build + compile the Bass module for one core's shard."""
    from contextlib import ExitStack

    nc = bacc.Bacc(
        "TRN2", target_bir_lowering=False, debug=False, num_devices=N_CORES
    )
    x_ap = nc.dram_tensor("x", [bpc, t, C], FP32, kind="ExternalInput").ap()
    out_ap = nc.dram_tensor("out", [bpc, t, C], FP32, kind="ExternalOutput").ap()
    with tile.TileContext(nc) as tc:
        with ExitStack() as ctx:
            _body(ctx, tc, out_ap, x_ap, r, bpc, t, dbg=dbg)
    nc.compile()
    return nc


def kernel(x, r_sigma):
    global LAST_RESULTS
    x = np.ascontiguousarray(np.asarray(x, dtype=np.float32))
    r = float(np.asarray(r_sigma).reshape(-1)[0])
    assert x.shape == (B, T, C), x.shape

    nc = build(r)
    in_maps = [
        {"x": np.ascontiguousarray(x[i * BPC : (i + 1) * BPC])}
        for i in range(N_CORES)
    ]
    trace = bool(int(os.environ.get("KERNEL_TRACE", "0")))
    res = run_bass_kernel_spmd(
        nc, in_maps, core_ids=list(range(N_CORES)), trace=trace
    )
    LAST_RESULTS = res
    out = np.concatenate([res.results[i]["out"] for i in range(N_CORES)], axis=0)
    return out.astype(np.float32)


# revision 28
# speedup vs baseline: 1.1595x; 1.1595x over previous
"""Gaussian-kernel attention for Trainium2 (Bass/Tile), 8-core data-parallel.

Computes out = x + K @ x with K = exp(-r * d2), d2[t,s] = ||x_t - x_s||^2,
per batch.  Decomposition used on-chip:

    d2 = sq_t + sq_s - 2*G          (G = X X^T, sq = rowwise |x|^2)
    K  = e_t * exp(2r*G) * e_s      (e_i = exp(-r*sq_i))
    out[t] = x[t] + e_t * sum_s [exp(2r*G - r*sq_s)][s,t] * x[s]

The e_s factor folds into the ACT exp as a per-partition bias, so mm2's
stationary operand is plain bf16 x.  Structure per core (4 batches):

  phase 1: all 4 batch prologues up front (x load, row stats, X^T via a
     DRAM round-trip DMA-xbar transpose) so the main loops never wait on
     the sync queue at batch boundaries.
  phase 2: one flat pipeline over (batch, t-block) pairs; per 1024-col
     t-block an s-loop of
       mm1 (G = X X^T, two row-group-packed N=512 matmuls, one s ahead)
       -> ACT exp (the only T^2 elementwise op, ~1.1us per tile; the
          critical-path engine, ~saturated)
       -> mm2 (outT += x^T A, two column-group-packed N=512 matmuls;
          chunk n lands on PSUM partitions n*64..n*64+63).
     The epilogue (outT -> out) PE-transposes 128-col chunks; those 8
     transposes + rescales drip through the NEXT block's s-loop so they
     ride the PE/DVE slack instead of stalling the ACT stream.

Sharding: pure data-parallel over batch B=32 -> 4 batches per core x 8 cores.
"""

import os
import sys

import numpy as np

sys.path.insert(0, "/opt/trn_rl_repo")

import concourse.bass as bass
import concourse.tile as tile
from concourse import bacc, masks, mybir
from concourse.bass_utils import run_bass_kernel_spmd

FP32 = mybir.dt.float32
BF16 = mybir.dt.bfloat16

B, T, C = 32, 2048, 64
N_CORES = 8
BPC = B // N_CORES  # batches per core

# Stashed by kernel() for the test harness (exec time etc.)
LAST_RESULTS = None


def _body(ctx, tc, out_ap, x_ap, r, bpc, t, dbg=False):
    """Emit the per-core kernel IR.

    out_ap/x_ap: DRAM APs of shape [bpc, t, C].
    r: python float (r_sigma value, baked as immediates).
    """
    nc = tc.nc

    def dump(name, sb_ap, dt=None):
        if not dbg:
            return
        d = nc.dram_tensor(
            name, list(sb_ap.shape), dt or sb_ap.dtype, kind="ExternalOutput"
        ).ap()
        nc.sync.dma_start(out=d, in_=sb_ap)

    nt = t // 128          # 128-row s/t blocks
    tblk = min(1024, t)    # t-block width processed per G tile (<= 2 psum banks)
    ntb = t // tblk        # outer t-block count
    kpb = tblk // 128      # 128-col sub-blocks per t-block

    exp2r = 2.0 * r

    # SBUF pools; per-batch tiles (x32/xt/stats) stay live all kernel
    xpool = ctx.enter_context(tc.tile_pool(name="x32", bufs=1))
    xxpool = ctx.enter_context(tc.tile_pool(name="xx", bufs=2))
    sqpool = ctx.enter_context(tc.tile_pool(name="sq", bufs=1))
    xbpool = ctx.enter_context(tc.tile_pool(name="xbp", bufs=2))
    xtpool = ctx.enter_context(tc.tile_pool(name="xt", bufs=1))
    idpool = ctx.enter_context(tc.tile_pool(name="idp", bufs=1))
    apool = ctx.enter_context(tc.tile_pool(name="a0", bufs=3))
    opool = ctx.enter_context(tc.tile_pool(name="osb", bufs=2))
    # PSUM (8 banks): G = [128, tblk] fp32 x2 bufs (4), P = [128, tblk] x1
    # (2), trp = [128, kpb, 128] bf16 x2 (2)
    gpool = ctx.enter_context(tc.tile_pool(name="gps", bufs=2, space="PSUM"))
    ppool = ctx.enter_context(tc.tile_pool(name="pps", bufs=1, space="PSUM"))
    tpool = ctx.enter_context(tc.tile_pool(name="tps", bufs=2, space="PSUM"))
    # DRAM scratch for the bf16 transpose round-trip
    dpool = ctx.enter_context(tc.tile_pool(name="dsc", bufs=2, space="DRAM"))

    ident = idpool.tile([128, 128], BF16)
    masks.make_identity(nc, ident[:])

    # Dependency-free dummy exp: pulls the ~1.3us ACT_TABLE_LOAD (lazily
    # inserted before the first ACTIVATE) into the startup dead time instead
    # of the head of the real exp stream.
    warm = idpool.tile([128, 32], FP32, name="warm")
    nc.gpsimd.memset(warm[:], 0.0)
    nc.scalar.activation(
        warm[:, 16:32], warm[:, 0:16], mybir.ActivationFunctionType.Exp
    )

    # Degree-6 polynomial fit of ev(s) = exp(-r*s) over the attainable range
    # of s = |x|^2 (chi^2_64; [0, 280] covers it with huge margin), weighted
    # for relative error.  Evaluated on the DVE so no ACT instruction ever
    # precedes the big exp stream (the scheduler hoists early ACT ops ahead
    # of it and stalls the whole pipeline on their inputs).
    s_grid = np.linspace(0.0, 280.0, 2048)
    vals = np.exp(-r * s_grid)
    cf = np.polyfit(s_grid, vals, 6, w=1.0 / vals)  # highest power first

    # ---- prologues (emitted just-in-time so their DMAs and ev-exps don't
    # head-of-line-block the sync/ACT queues at startup) ----
    x32s, sqs, evs, msqs, xts = [], [], [], [], []

    def prologue(b):
        xb_dram = x_ap[b].rearrange("(k p) c -> p k c", p=128)   # [128, nt, C]

        x32 = xpool.tile([128, nt, C], FP32, tag=f"x32_{b}")
        # batch 0's load must reach SBUF ASAP; later batches' loads go on
        # the idle gpsimd (SWDGE) queue so the scheduler can't hoist them
        # ahead of it on the sync queue.
        if b == 0:
            nc.sync.dma_start(out=x32[:], in_=xb_dram)
        else:
            nc.gpsimd.dma_start(out=x32[:], in_=xb_dram)
        xt = xtpool.tile([128, t + nt * C], BF16, tag=f"xt{b}")
        # xt[c, tt] = xt[64+c, tt] = x[tt, c] for c < 64; cols [t, t+nt*C)
        # hold bf16 x with s on partitions (mm2's stationary operand).

        def stats():
            # the squares run on the otherwise-idle GPSIMD engine, off the
            # DVE queue (which serializes the startup transpose-copies)
            xx = xxpool.tile([128, nt, C], FP32, name="xx")
            nc.gpsimd.tensor_mul(xx[:], x32[:], x32[:])
            sq = sqpool.tile([128, nt], FP32, name="sq", tag=f"sq{b}")
            nc.vector.tensor_reduce(
                sq[:], xx[:], axis=mybir.AxisListType.X, op=mybir.AluOpType.add
            )
            # per-partition exp bias: -r*|x_s|^2 (folds e_s into the exp)
            msq = sqpool.tile([128, nt], FP32, name="msq", tag=f"msq{b}")
            nc.vector.tensor_scalar_mul(msq[:], sq[:], -r)
            # ev = exp(-r*sq) via DVE Horner (8 tiny ops, no ACT involved)
            t0 = sqpool.tile([128, nt], FP32, name="t0", tag=f"pa{b % 2}")
            t1 = sqpool.tile([128, nt], FP32, name="t1", tag=f"pb{b % 2}")
            nc.vector.tensor_scalar(
                t0[:], sq[:], 0.0, float(cf[0]),
                op0=mybir.AluOpType.mult, op1=mybir.AluOpType.add,
            )
            cur, nxt = t0, t1
            for a in (0.0, cf[1], cf[2], cf[3], cf[4], cf[5]):
                nc.vector.scalar_tensor_tensor(
                    nxt[:], in0=cur[:], scalar=float(a), in1=sq[:],
                    op0=mybir.AluOpType.add, op1=mybir.AluOpType.mult,
                )
                cur, nxt = nxt, cur
            ev = sqpool.tile([128, nt], FP32, name="ev", tag=f"ev{b}")
            nc.vector.tensor_scalar_add(ev[:], cur[:], float(cf[6]))
            return sq, msq, ev

        if b == 0:
            # Batch 0 gates the whole exp stream, and its DMA-xbar round
            # trip sits ~10us behind DVE stats + queue hoisting.  Hybrid:
            # build X^T for k-blocks 0..7 (needed by the first t-window and
            # the early s-weights) with PE transposes + DVE copies, and let
            # blocks 8..15 arrive via the DMA round trip (pure DMA work,
            # reading the bf16 staging copy; not needed for ~15us).
            xb16 = xbpool.tile([128, nt, C], BF16, tag="xb16")
            nc.vector.tensor_copy(xb16[:], x32[:])
            xbd0 = dpool.tile([t // 2, 2 * C], BF16, name="xbd0")
            xbdv = xbd0.rearrange("(k p) c -> p k c", p=128)
            nc.sync.dma_start(out=xbdv[:, :, 0:C], in_=xb16[:, nt // 2 :])
            nc.sync.dma_start(out=xbdv[:, :, C : 2 * C], in_=xb16[:, nt // 2 :])
            nc.sync.dma_start_transpose(out=xt[:, t // 2 : t], in_=xbd0[:])
            for k in range(nt // 2):
                trq = tpool.tile([128, 128], BF16, name="trpj")
                nc.tensor.transpose(trq[0:64, :], xb16[:, k], ident[:])
                nc.vector.tensor_copy(
                    xt[0:64, k * 128 : (k + 1) * 128], trq[0:64, :]
                )
                nc.vector.tensor_copy(
                    xt[64:128, k * 128 : (k + 1) * 128], trq[0:64, :]
                )
            sq, msq, ev = stats()
            nc.vector.tensor_copy(
                xt[:, t : t + nt * C].rearrange("p (k c) -> p k c", c=C),
                xb16[:],
            )
        else:
            sq, msq, ev = stats()
            # bf16 x written TWICE side by side so one full-width DMA-xbar
            # transpose yields X^T duplicated on both partition halves (row
            # groups for packed mm1); hidden under earlier batches' compute.
            xbp = xbpool.tile([128, nt, 2 * C], BF16, name="xbp")
            nc.vector.tensor_copy(xbp[:, :, 0:C], x32[:])
            nc.vector.tensor_copy(xbp[:, :, C : 2 * C], x32[:])
            xbd = dpool.tile([t, 2 * C], BF16, name="xbd")
            nc.sync.dma_start(
                out=xbd.rearrange("(k p) c -> p k c", p=128), in_=xbp[:]
            )
            nc.sync.dma_start_transpose(out=xt[:, 0:t], in_=xbd[:])
            nc.vector.tensor_copy(
                xt[:, t : t + nt * C].rearrange("p (k c) -> p k c", c=C),
                xbp[:, :, 0:C],
            )
        x32s.append(x32)
        sqs.append(sq)
        evs.append(ev)
        msqs.append(msq)
        xts.append(xt)
        if dbg and b == 0:
            dump("dbg_sq", sq[:])

    # ---- main: flat pipeline over (batch, t-block) ----
    blocks = [(b, ti) for b in range(bpc) for ti in range(ntb)]

    def mm1(b, ti, s):
        xt = xts[b]
        g = gpool.tile([128, tblk], FP32, name="g_ps", tag="g")
        for n in range(2):
            base = 64 * n
            nc.tensor.matmul(
                g[:, n * 512 : (n + 1) * 512],
                lhsT=xt[base : base + 64, s * 128 : (s + 1) * 128],
                rhs=xt[
                    base : base + 64,
                    ti * tblk + n * 512 : ti * tblk + (n + 1) * 512,
                ],
                start=True,
                stop=True,
                tile_position=(base, 0),
            )
        return g

    pend = None  # previous block's epilogue: dict(ot, osb, b, ti)

    def drip(j):
        ot = pend["ot"]
        pb, pti = pend["b"], pend["ti"]
        # per-j trp tiles from a 2-buffer pool: adjacent transposes land in
        # different PSUM banks, so transpose j+1 (start=True clears its
        # bank) doesn't serialize against j's reader.
        trp = tpool.tile([128, 128], BF16, name="trpj")
        nc.tensor.transpose(trp[:], ot[:, j * 128 : (j + 1) * 128], ident[:])
        k = pti * kpb + j
        half = (j // (kpb // 2)) * 64
        nc.vector.scalar_tensor_tensor(
            pend["osb"][:, j],
            in0=trp[:, half : half + 64],
            scalar=evs[pb][:, k : k + 1],
            in1=x32s[pb][:, k],
            op0=mybir.AluOpType.mult,
            op1=mybir.AluOpType.add,
        )

    def flush():
        pb, pti = pend["b"], pend["ti"]
        ob_dram = out_ap[pb].rearrange("(k p) c -> p k c", p=128)
        nc.sync.dma_start(
            out=ob_dram[:, pti * kpb : (pti + 1) * kpb], in_=pend["osb"][:]
        )

    def tail_stt(osb, trp, tb, tti, j):
        k = tti * kpb + j
        half = (j // (kpb // 2)) * 64
        nc.vector.scalar_tensor_tensor(
            osb[:, j],
            in0=trp[:, half : half + 64],
            scalar=evs[tb][:, k : k + 1],
            in1=x32s[tb][:, k],
            op0=mybir.AluOpType.mult,
            op1=mybir.AluOpType.add,
        )

    prologue(0)
    prologue(1)
    g_cur = mm1(*blocks[0], 0)
    for bi, (b, ti) in enumerate(blocks):
        if bi in (1, 2) and bi + 1 < bpc:
            prologue(bi + 1)
        msq, xt = msqs[b], xts[b]
        yw = xt[:, t : t + nt * C].rearrange("p (k c) -> p k c", c=C)
        p_ps = ppool.tile([128, tblk], FP32)

        for s in range(nt):
            if s + 1 < nt:
                g_next = mm1(b, ti, s + 1)
            elif bi + 1 < len(blocks):
                g_next = mm1(*blocks[bi + 1], 0)
            else:
                g_next = None
            a0 = apool.tile([128, tblk], BF16)
            nc.scalar.activation(
                a0[:],
                g_cur[:],
                mybir.ActivationFunctionType.Exp,
                scale=exp2r,
                bias=msq[:, s : s + 1],
            )
            if dbg and b == 0 and ti == 0 and s == 0:
                gsb = xxpool.tile([128, tblk], FP32, tag="gdump")
                nc.vector.tensor_copy(gsb[:], g_cur[:])
                dump("dbg_g00", gsb[:])
                dump("dbg_a00", a0[:])
            # mm2: two concurrent column-group matmuls; chunk n -> PSUM
            # partitions [n*64, n*64+64), cols [n*512, n*512+512).
            for n in range(2):
                nc.tensor.matmul(
                    p_ps[n * 64 : (n + 1) * 64, n * 512 : (n + 1) * 512],
                    lhsT=yw[:, s],
                    rhs=a0[:, n * 512 : (n + 1) * 512],
                    start=(s == 0),
                    stop=(s == nt - 1),
                    skip_group_check=True,
                    tile_position=(0, n * 64),
                )
            # drip the previous block's epilogue through this block's slack
            if pend is not None:
                if s % 2 == 0 and s // 2 < kpb:
                    drip(s // 2)
                if s == 2 * kpb - 1:
                    flush()
                    pend = None
            g_cur = g_next

        if bi == len(blocks) - 1:
            # final block: its epilogue is fully exposed — pipeline it at
            # 128-col granularity (chunk CAST -> PE transpose -> rescale)
            # instead of a serial bulk CAST followed by the drip chain.
            otl = opool.tile([128, tblk], BF16, tag="ot")
            osb = opool.tile([128, kpb, C], FP32, name="osb", tag="osb")
            ob_dram = out_ap[b].rearrange("(k p) c -> p k c", p=128)
            trps = []
            for j in range(kpb):
                nc.vector.tensor_copy(
                    otl[:, j * 128 : (j + 1) * 128],
                    p_ps[:, j * 128 : (j + 1) * 128],
                )
                trp = tpool.tile([128, 128], BF16, name="trpj")
                nc.tensor.transpose(
                    trp[:], otl[:, j * 128 : (j + 1) * 128], ident[:]
                )
                trps.append(trp)
                if j >= 1:
                    tail_stt(osb, trps[j - 1], b, ti, j - 1)
            tail_stt(osb, trps[kpb - 1], b, ti, kpb - 1)
            nc.sync.dma_start(
                out=ob_dram[:, ti * kpb : (ti + 1) * kpb], in_=osb[:]
            )
        else:
            # stage this block's epilogue
            ot = opool.tile([128, tblk], BF16, tag="ot")
            nc.vector.tensor_copy(ot[:], p_ps[:])
            osb = opool.tile([128, kpb, C], FP32, name="osb", tag="osb")
            pend = {"ot": ot, "osb": osb, "b": b, "ti": ti}


def build(r, bpc=BPC, t=T, dbg=False):
    """Build + compile the Bass module for one core's shard."""
    from contextlib import ExitStack

    nc = bacc.Bacc(
        "TRN2", target_bir_lowering=False, debug=False, num_devices=N_CORES
    )
    x_ap = nc.dram_tensor("x", [bpc, t, C], FP32, kind="ExternalInput").ap()
    out_ap = nc.dram_tensor("out", [bpc, t, C], FP32, kind="ExternalOutput").ap()
    with tile.TileContext(nc) as tc:
        with ExitStack() as ctx:
            _body(ctx, tc, out_ap, x_ap, r, bpc, t, dbg=dbg)
    nc.compile()
    return nc


def kernel(x, r_sigma):
    global LAST_RESULTS
    x = np.ascontiguousarray(np.asarray(x, dtype=np.float32))
    r = float(np.asarray(r_sigma).reshape(-1)[0])
    assert x.shape == (B, T, C), x.shape

    nc = build(r)
    in_maps = [
        {"x": np.ascontiguousarray(x[i * BPC : (i + 1) * BPC])}
        for i in range(N_CORES)
    ]
    trace = bool(int(os.environ.get("KERNEL_TRACE", "0")))
    res = run_bass_kernel_spmd(
        nc, in_maps, core_ids=list(range(N_CORES)), trace=trace
    )
    LAST_RESULTS = res
    out = np.concatenate([res.results[i]["out"] for i in range(N_CORES)], axis=0)
    return out.astype(np.float32)


# revision 30
# speedup vs baseline: 1.1861x; 1.0229x over previous
"""Gaussian-kernel attention for Trainium2 (Bass/Tile), 8-core data-parallel.

Computes out = x + K @ x with K = exp(-r * d2), d2[t,s] = ||x_t - x_s||^2,
per batch.  Decomposition used on-chip:

    d2 = sq_t + sq_s - 2*G          (G = X X^T, sq = rowwise |x|^2)
    K  = e_t * exp(2r*G) * e_s      (e_i = exp(-r*sq_i))
    out[t] = x[t] + e_t * sum_s [exp(2r*G - r*sq_s)][s,t] * x[s]

The e_s factor folds into the ACT exp as a per-partition bias, so mm2's
stationary operand is plain bf16 x.  Structure per core (4 batches):

  phase 1: all 4 batch prologues up front (x load, row stats, X^T via a
     DRAM round-trip DMA-xbar transpose) so the main loops never wait on
     the sync queue at batch boundaries.
  phase 2: one flat pipeline over (batch, t-block) pairs; per 1024-col
     t-block an s-loop of
       mm1 (G = X X^T, two row-group-packed N=512 matmuls, one s ahead)
       -> ACT exp (the only T^2 elementwise op, ~1.1us per tile; the
          critical-path engine, ~saturated)
       -> mm2 (outT += x^T A, two column-group-packed N=512 matmuls;
          chunk n lands on PSUM partitions n*64..n*64+63).
     The epilogue (outT -> out) PE-transposes 128-col chunks; those 8
     transposes + rescales drip through the NEXT block's s-loop so they
     ride the PE/DVE slack instead of stalling the ACT stream.

Sharding: pure data-parallel over batch B=32 -> 4 batches per core x 8 cores.
"""

import os
import sys

import numpy as np

sys.path.insert(0, "/opt/trn_rl_repo")

import concourse.bass as bass
import concourse.tile as tile
from concourse import bacc, masks, mybir
from concourse.bass_utils import run_bass_kernel_spmd

FP32 = mybir.dt.float32
BF16 = mybir.dt.bfloat16

B, T, C = 32, 2048, 64
N_CORES = 8
BPC = B // N_CORES  # batches per core

# Stashed by kernel() for the test harness (exec time etc.)
LAST_RESULTS = None


def _body(ctx, tc, out_ap, x_ap, r, bpc, t, dbg=False):
    """Emit the per-core kernel IR.

    out_ap/x_ap: DRAM APs of shape [bpc, t, C].
    r: python float (r_sigma value, baked as immediates).
    """
    nc = tc.nc

    def dump(name, sb_ap, dt=None):
        if not dbg:
            return
        d = nc.dram_tensor(
            name, list(sb_ap.shape), dt or sb_ap.dtype, kind="ExternalOutput"
        ).ap()
        nc.sync.dma_start(out=d, in_=sb_ap)

    nt = t // 128          # 128-row s/t blocks
    tblk = min(1024, t)    # t-block width processed per G tile (<= 2 psum banks)
    ntb = t // tblk        # outer t-block count
    kpb = tblk // 128      # 128-col sub-blocks per t-block

    exp2r = 2.0 * r

    # SBUF pools; per-batch tiles (x32/xt/stats) stay live all kernel
    xpool = ctx.enter_context(tc.tile_pool(name="x32", bufs=1))
    xxpool = ctx.enter_context(tc.tile_pool(name="xx", bufs=2))
    sqpool = ctx.enter_context(tc.tile_pool(name="sq", bufs=1))
    xbpool = ctx.enter_context(tc.tile_pool(name="xbp", bufs=2))
    xtpool = ctx.enter_context(tc.tile_pool(name="xt", bufs=1))
    idpool = ctx.enter_context(tc.tile_pool(name="idp", bufs=1))
    apool = ctx.enter_context(tc.tile_pool(name="a0", bufs=3))
    opool = ctx.enter_context(tc.tile_pool(name="osb", bufs=2))
    # PSUM (8 banks): G = [128, tblk] fp32 x2 bufs (4), P = [128, tblk] x1
    # (2), trp = [128, kpb, 128] bf16 x2 (2)
    gpool = ctx.enter_context(tc.tile_pool(name="gps", bufs=2, space="PSUM"))
    ppool = ctx.enter_context(tc.tile_pool(name="pps", bufs=1, space="PSUM"))
    tpool = ctx.enter_context(tc.tile_pool(name="tps", bufs=2, space="PSUM"))
    # DRAM scratch for the bf16 transpose round-trip
    dpool = ctx.enter_context(tc.tile_pool(name="dsc", bufs=2, space="DRAM"))

    ident = idpool.tile([128, 128], BF16)
    masks.make_identity(nc, ident[:])

    # Dependency-free dummy exp: pulls the ~1.3us ACT_TABLE_LOAD (lazily
    # inserted before the first ACTIVATE) into the startup dead time instead
    # of the head of the real exp stream.
    warm = idpool.tile([128, 32], FP32, name="warm")
    nc.gpsimd.memset(warm[:], 0.0)
    nc.scalar.activation(
        warm[:, 16:32], warm[:, 0:16], mybir.ActivationFunctionType.Exp
    )

    # Degree-6 polynomial fit of ev(s) = exp(-r*s) over the attainable range
    # of s = |x|^2 (chi^2_64; [0, 280] covers it with huge margin), weighted
    # for relative error.  Evaluated on the DVE so no ACT instruction ever
    # precedes the big exp stream (the scheduler hoists early ACT ops ahead
    # of it and stalls the whole pipeline on their inputs).
    s_grid = np.linspace(0.0, 280.0, 2048)
    vals = np.exp(-r * s_grid)
    cf = np.polyfit(s_grid, vals, 6, w=1.0 / vals)  # highest power first

    # ---- prologues (emitted just-in-time so their DMAs and ev-exps don't
    # head-of-line-block the sync/ACT queues at startup) ----
    x32s, sqs, evs, msqs, xts = [], [], [], [], []

    def prologue(b):
        xb_dram = x_ap[b].rearrange("(k p) c -> p k c", p=128)   # [128, nt, C]

        x32 = xpool.tile([128, nt, C], FP32, tag=f"x32_{b}")
        # batch 0's load must reach SBUF ASAP; later batches' loads go on
        # the idle gpsimd (SWDGE) queue so the scheduler can't hoist them
        # ahead of it on the sync queue.
        if b == 0:
            nc.sync.dma_start(out=x32[:], in_=xb_dram)
        else:
            nc.gpsimd.dma_start(out=x32[:], in_=xb_dram)
        xt = xtpool.tile([128, t + nt * C], BF16, tag=f"xt{b}")
        # xt[c, tt] = xt[64+c, tt] = x[tt, c] for c < 64; cols [t, t+nt*C)
        # hold bf16 x with s on partitions (mm2's stationary operand).

        def stats():
            xx = xxpool.tile([128, nt, C], FP32, name="xx")
            nc.vector.tensor_mul(xx[:], x32[:], x32[:])
            sq = sqpool.tile([128, nt], FP32, name="sq", tag=f"sq{b}")
            nc.vector.tensor_reduce(
                sq[:], xx[:], axis=mybir.AxisListType.X, op=mybir.AluOpType.add
            )
            # per-partition exp bias: -r*|x_s|^2 (folds e_s into the exp)
            msq = sqpool.tile([128, nt], FP32, name="msq", tag=f"msq{b}")
            nc.vector.tensor_scalar_mul(msq[:], sq[:], -r)
            # ev = exp(-r*sq) via DVE Horner (8 tiny ops, no ACT involved)
            t0 = sqpool.tile([128, nt], FP32, name="t0", tag=f"pa{b % 2}")
            t1 = sqpool.tile([128, nt], FP32, name="t1", tag=f"pb{b % 2}")
            nc.vector.tensor_scalar(
                t0[:], sq[:], 0.0, float(cf[0]),
                op0=mybir.AluOpType.mult, op1=mybir.AluOpType.add,
            )
            cur, nxt = t0, t1
            for a in (0.0, cf[1], cf[2], cf[3], cf[4], cf[5]):
                nc.vector.scalar_tensor_tensor(
                    nxt[:], in0=cur[:], scalar=float(a), in1=sq[:],
                    op0=mybir.AluOpType.add, op1=mybir.AluOpType.mult,
                )
                cur, nxt = nxt, cur
            ev = sqpool.tile([128, nt], FP32, name="ev", tag=f"ev{b}")
            nc.vector.tensor_scalar_add(ev[:], cur[:], float(cf[6]))
            return sq, msq, ev

        if b == 0:
            # Batch 0 gates the whole exp stream, and its DMA-xbar round
            # trip sits ~10us behind DVE stats + queue hoisting.  Hybrid:
            # build X^T for k-blocks 0..7 (needed by the first t-window and
            # the early s-weights) with PE transposes + DVE copies, and let
            # blocks 8..15 arrive via the DMA round trip (pure DMA work,
            # reading the bf16 staging copy; not needed for ~15us).
            xb16 = xbpool.tile([128, nt, C], BF16, tag="xb16")
            nc.vector.tensor_copy(xb16[:], x32[:])
            xbd0 = dpool.tile([t // 2, 2 * C], BF16, name="xbd0")
            xbdv = xbd0.rearrange("(k p) c -> p k c", p=128)
            nc.sync.dma_start(out=xbdv[:, :, 0:C], in_=xb16[:, nt // 2 :])
            nc.sync.dma_start(out=xbdv[:, :, C : 2 * C], in_=xb16[:, nt // 2 :])
            nc.sync.dma_start_transpose(out=xt[:, t // 2 : t], in_=xbd0[:])
            for k in range(nt // 2):
                trq = tpool.tile([128, 128], BF16, name="trpj")
                nc.tensor.transpose(trq[0:64, :], xb16[:, k], ident[:])
                nc.vector.tensor_copy(
                    xt[0:64, k * 128 : (k + 1) * 128], trq[0:64, :]
                )
                nc.vector.tensor_copy(
                    xt[64:128, k * 128 : (k + 1) * 128], trq[0:64, :]
                )
            sq, msq, ev = stats()
            nc.vector.tensor_copy(
                xt[:, t : t + nt * C].rearrange("p (k c) -> p k c", c=C),
                xb16[:],
            )
        else:
            sq, msq, ev = stats()
            # bf16 x written TWICE side by side so one full-width DMA-xbar
            # transpose yields X^T duplicated on both partition halves (row
            # groups for packed mm1); hidden under earlier batches' compute.
            xbp = xbpool.tile([128, nt, 2 * C], BF16, name="xbp")
            nc.vector.tensor_copy(xbp[:, :, 0:C], x32[:])
            nc.vector.tensor_copy(xbp[:, :, C : 2 * C], x32[:])
            xbd = dpool.tile([t, 2 * C], BF16, name="xbd")
            nc.sync.dma_start(
                out=xbd.rearrange("(k p) c -> p k c", p=128), in_=xbp[:]
            )
            nc.sync.dma_start_transpose(out=xt[:, 0:t], in_=xbd[:])
            nc.vector.tensor_copy(
                xt[:, t : t + nt * C].rearrange("p (k c) -> p k c", c=C),
                xbp[:, :, 0:C],
            )
        x32s.append(x32)
        sqs.append(sq)
        evs.append(ev)
        msqs.append(msq)
        xts.append(xt)
        if dbg and b == 0:
            dump("dbg_sq", sq[:])

    # ---- main: flat pipeline over (batch, t-block) ----
    blocks = [(b, ti) for b in range(bpc) for ti in range(ntb)]

    def mm1(b, ti, s):
        xt = xts[b]
        g = gpool.tile([128, tblk], FP32, name="g_ps", tag="g")
        for n in range(2):
            base = 64 * n
            nc.tensor.matmul(
                g[:, n * 512 : (n + 1) * 512],
                lhsT=xt[base : base + 64, s * 128 : (s + 1) * 128],
                rhs=xt[
                    base : base + 64,
                    ti * tblk + n * 512 : ti * tblk + (n + 1) * 512,
                ],
                start=True,
                stop=True,
                tile_position=(base, 0),
            )
        return g

    pend = None  # previous block's epilogue: dict(ot, osb, b, ti)

    def drip(j):
        ot = pend["ot"]
        pb, pti = pend["b"], pend["ti"]
        # per-j trp tiles from a 2-buffer pool: adjacent transposes land in
        # different PSUM banks, so transpose j+1 (start=True clears its
        # bank) doesn't serialize against j's reader.
        trp = tpool.tile([128, 128], BF16, name="trpj")
        nc.tensor.transpose(trp[:], ot[:, j * 128 : (j + 1) * 128], ident[:])
        k = pti * kpb + j
        half = (j // (kpb // 2)) * 64
        nc.vector.scalar_tensor_tensor(
            pend["osb"][:, j],
            in0=trp[:, half : half + 64],
            scalar=evs[pb][:, k : k + 1],
            in1=x32s[pb][:, k],
            op0=mybir.AluOpType.mult,
            op1=mybir.AluOpType.add,
        )

    def flush():
        pb, pti = pend["b"], pend["ti"]
        ob_dram = out_ap[pb].rearrange("(k p) c -> p k c", p=128)
        nc.sync.dma_start(
            out=ob_dram[:, pti * kpb : (pti + 1) * kpb], in_=pend["osb"][:]
        )

    def tail_stt(osb, trp, tb, tti, j):
        k = tti * kpb + j
        half = (j // (kpb // 2)) * 64
        nc.vector.scalar_tensor_tensor(
            osb[:, j],
            in0=trp[:, half : half + 64],
            scalar=evs[tb][:, k : k + 1],
            in1=x32s[tb][:, k],
            op0=mybir.AluOpType.mult,
            op1=mybir.AluOpType.add,
        )

    prologue(0)
    prologue(1)
    g_cur = mm1(*blocks[0], 0)
    for bi, (b, ti) in enumerate(blocks):
        if bi in (1, 2) and bi + 1 < bpc:
            prologue(bi + 1)
        msq, xt = msqs[b], xts[b]
        yw = xt[:, t : t + nt * C].rearrange("p (k c) -> p k c", c=C)
        p_ps = ppool.tile([128, tblk], FP32)

        for s in range(nt):
            if s + 1 < nt:
                g_next = mm1(b, ti, s + 1)
            elif bi + 1 < len(blocks):
                g_next = mm1(*blocks[bi + 1], 0)
            else:
                g_next = None
            a0 = apool.tile([128, tblk], BF16)
            nc.scalar.activation(
                a0[:],
                g_cur[:],
                mybir.ActivationFunctionType.Exp,
                scale=exp2r,
                bias=msq[:, s : s + 1],
            )
            if dbg and b == 0 and ti == 0 and s == 0:
                gsb = xxpool.tile([128, tblk], FP32, tag="gdump")
                nc.vector.tensor_copy(gsb[:], g_cur[:])
                dump("dbg_g00", gsb[:])
                dump("dbg_a00", a0[:])
            # mm2: two concurrent column-group matmuls; chunk n -> PSUM
            # partitions [n*64, n*64+64), cols [n*512, n*512+512).
            for n in range(2):
                nc.tensor.matmul(
                    p_ps[n * 64 : (n + 1) * 64, n * 512 : (n + 1) * 512],
                    lhsT=yw[:, s],
                    rhs=a0[:, n * 512 : (n + 1) * 512],
                    start=(s == 0),
                    stop=(s == nt - 1),
                    skip_group_check=True,
                    tile_position=(0, n * 64),
                )
            # drip the previous block's epilogue through this block's slack
            if pend is not None:
                if s % 2 == 0 and s // 2 < kpb:
                    drip(s // 2)
                if s == 2 * kpb - 1:
                    flush()
                    pend = None
            g_cur = g_next

        if bi == len(blocks) - 1:
            # final block: its epilogue is fully exposed — pipeline it at
            # 128-col granularity (chunk CAST -> PE transpose -> rescale)
            # instead of a serial bulk CAST followed by the drip chain.
            otl = opool.tile([128, tblk], BF16, tag="ot")
            osb = opool.tile([128, kpb, C], FP32, name="osb", tag="osb")
            ob_dram = out_ap[b].rearrange("(k p) c -> p k c", p=128)
            trps = []
            for j in range(kpb):
                # tail chunk-casts run on the ACT engine (idle once the exp
                # stream ends) so the DVE only carries the rescales here
                nc.scalar.copy(
                    otl[:, j * 128 : (j + 1) * 128],
                    p_ps[:, j * 128 : (j + 1) * 128],
                )
                trp = tpool.tile([128, 128], BF16, name="trpj")
                nc.tensor.transpose(
                    trp[:], otl[:, j * 128 : (j + 1) * 128], ident[:]
                )
                trps.append(trp)
                if j >= 1:
                    tail_stt(osb, trps[j - 1], b, ti, j - 1)
            tail_stt(osb, trps[kpb - 1], b, ti, kpb - 1)
            nc.sync.dma_start(
                out=ob_dram[:, ti * kpb : (ti + 1) * kpb], in_=osb[:]
            )
        else:
            # stage this block's epilogue
            ot = opool.tile([128, tblk], BF16, tag="ot")
            nc.vector.tensor_copy(ot[:], p_ps[:])
            osb = opool.tile([128, kpb, C], FP32, name="osb", tag="osb")
            pend = {"ot": ot, "osb": osb, "b": b, "ti": ti}


def build(r, bpc=BPC, t=T, dbg=False):
    """Build + compile the Bass module for one core's shard."""
    from contextlib import ExitStack

    nc = bacc.Bacc(
        "TRN2", target_bir_lowering=False, debug=False, num_devices=N_CORES
    )
    x_ap = nc.dram_tensor("x", [bpc, t, C], FP32, kind="ExternalInput").ap()
    out_ap = nc.dram_tensor("out", [bpc, t, C], FP32, kind="ExternalOutput").ap()
    with tile.TileContext(nc) as tc:
        with ExitStack() as ctx:
            _body(ctx, tc, out_ap, x_ap, r, bpc, t, dbg=dbg)
    nc.compile()
    return nc


def kernel(x, r_sigma):
    global LAST_RESULTS
    x = np.ascontiguousarray(np.asarray(x, dtype=np.float32))
    r = float(np.asarray(r_sigma).reshape(-1)[0])
    assert x.shape == (B, T, C), x.shape

    nc = build(r)
    in_maps = [
        {"x": np.ascontiguousarray(x[i * BPC : (i + 1) * BPC])}
        for i in range(N_CORES)
    ]
    trace = bool(int(os.environ.get("KERNEL_TRACE", "0")))
    res = run_bass_kernel_spmd(
        nc, in_maps, core_ids=list(range(N_CORES)), trace=trace
    )
    LAST_RESULTS = res
    out = np.concatenate([res.results[i]["out"] for i in range(N_CORES)], axis=0)
    return out.astype(np.float32)
